# revision 30
# baseline (speedup 1.0000x reference)
"""Trainium2 Bass kernel for nn_MHA_43095701848407.

MHA forward: qkv = x @ W_qkv, RoPE on q/k, causal softmax attention,
y @ W_proj.  B=4, T=2048, C=2048, 16 heads, head_dim=128.

Sharding (8 cores): tensor-parallel over heads (4 shards x 4 heads) x
data-parallel over batch (2 groups x 2 batches).  core = group*4 + shard.

v3 design: all matmul operands bf16; q^T/k^T/v SBUF-resident per batch;
per-batch pipeline qkv->attn->proj with x prefetch.  PE wall per matmul
is ~N/2.4GHz + 3ns (weight loads hidden for bf16), so the kernel
minimizes streamed columns and instruction count: RoPE rotate-half via
partition-strided SBUF->SBUF DMA (off the PE), v transposed to natural
layout with one wide DMA-transpose per chunk-slab, 2-bank PSUM tiles
with merged single-instruction exp per key block, causal mask as a
triangular-mask multiply on DVE, softmax 1/l broadcast via PE outer
product, proj loops reordered.  v chunks are computed before q/k so the
v transposes drain during qk compute (no bubble into attention).
Host sums the 4 head-shard bf16 partials per batch in f32.

Self-contained: shapes/sharding hardcoded; inputs full-size numpy arrays.
"""

import math
import os
import sys
import types

import ml_dtypes
import numpy as np

import concourse.bass as bass
import concourse.mybir as mybir
import concourse.tile as tile
from concourse import bacc
from concourse.bass_utils import run_bass_kernel_spmd

F32 = mybir.dt.float32
BF16 = mybir.dt.bfloat16
AF = mybir.ActivationFunctionType
ALU = mybir.AluOpType
NPBF = ml_dtypes.bfloat16

# Problem shape (hardcoded per contract)
B, T, C = 4, 2048, 2048
H, HD = 16, 128
NCORES = 8
BGROUPS, HSHARDS = 2, 4  # batch groups x head shards
B_LOC = B // BGROUPS  # 2 batches per core
H_LOC = H // HSHARDS  # 4 heads per core
FQK = H_LOC * HD  # 512 features for q (and for k)
FV = H_LOC * HD  # 512 for v
NCH = 12  # qkv feature chunks of 128 (4 q + 4 k + 4 v)
# v chunks (8..11) first so their DMA transposes drain during qk compute;
# qk interleaved q0,k0,q1,k1,... so early heads are ready first.
CHUNK_ORDER = [8, 9, 10, 11, 0, 4, 1, 5, 2, 6, 3, 7]
KO = C // 128  # 16 contraction chunks
KOG = 4  # x DMA granularity: 4 ko chunks per transfer
TSLAB = 1024
NSLAB = T // TSLAB  # 2 t-slabs per batch
QT = 1024  # attention q tile
NQT = T // QT  # 2 q tiles
NKBT = QT // 128  # 8 key blocks per q tile width
SCALE = 1.0 / math.sqrt(HD)

_CACHED = {}


def _install_ntff_hook():
    """Register the axon NTFF profile hook (container's antenv lacks it)."""
    if "antenv.axon_hooks" in sys.modules:
        return
    try:
        mod = types.ModuleType("antenv.axon_hooks")
        holder = [None]
        mod.set_axon_ntff_profile_hook = lambda h: holder.__setitem__(0, h)
        mod.get_axon_ntff_profile_hook = lambda: holder[0]
        sys.modules["antenv.axon_hooks"] = mod
        import antenv

        antenv.axon_hooks = mod
        if "/root/.axon_site" not in sys.path:
            sys.path.insert(0, "/root/.axon_site")
        from trn_agent_boot.trn_boot import _ntff_profile_via_ctypes

        mod.set_axon_ntff_profile_hook(
            _ntff_profile_via_ctypes("/opt/axon/libaxon_pjrt.so")
        )
    except Exception:
        sys.modules.pop("antenv.axon_hooks", None)


def build_nc():
    nc = bacc.Bacc("TRN2", target_bir_lowering=False, debug=False)

    x_t = nc.dram_tensor("x_t", [B_LOC, C, T], BF16, kind="ExternalInput").ap()
    w_qkv = nc.dram_tensor("w_qkv", [128, NCH, KO, 128], BF16,
                           kind="ExternalInput").ap()
    w_proj = nc.dram_tensor("w_proj", [FV, C], BF16, kind="ExternalInput").ap()
    cos_t = nc.dram_tensor("cos_t", [HD, T], BF16, kind="ExternalInput").ap()
    sin_t = nc.dram_tensor("sin_t", [HD, T], BF16, kind="ExternalInput").ap()
    ones_col = nc.dram_tensor("ones_col", [128, 1], BF16, kind="ExternalInput").ap()
    ones_row = nc.dram_tensor("ones_row", [1, 128], BF16, kind="ExternalInput").ap()
    tri = nc.dram_tensor("tri", [128, 128], BF16, kind="ExternalInput").ap()
    out_t = nc.dram_tensor("out_t", [B_LOC, C, T], BF16, kind="ExternalOutput").ap()

    with tile.TileContext(nc) as tc:
        with nc.allow_low_precision(reason="bf16 matmuls by design; tol 2e-2"):
            _emit(nc, tc, x_t, w_qkv, w_proj, cos_t, sin_t, ones_col,
                  ones_row, tri, out_t)
    nc.compile()
    return nc


def _emit(nc, tc, x_t, w_qkv, w_proj, cos_t, sin_t, ones_col, ones_row,
          tri, out_t):
    with (
        tc.tile_pool(name="consts", bufs=1) as consts,
        tc.tile_pool(name="wq", bufs=1) as wqpool,
        tc.tile_pool(name="wp", bufs=4) as wppool,
        tc.tile_pool(name="qkres", bufs=1) as qkres,
        tc.tile_pool(name="vres", bufs=1) as vres,
        tc.tile_pool(name="yres", bufs=1) as yres,
        tc.tile_pool(name="xpool", bufs=3) as xpool,
        tc.tile_pool(name="rawpool", bufs=2) as rawpool,
        tc.tile_pool(name="shufpool", bufs=2) as shufpool,
        tc.tile_pool(name="vtpool", bufs=2) as vtpool,
        tc.tile_pool(name="ppool", bufs=4) as ppool,
        tc.tile_pool(name="nfpool", bufs=1) as nfpool,
        tc.tile_pool(name="nbpool", bufs=1) as nbpool,
        tc.tile_pool(name="bcpool", bufs=2) as bcpool,
        tc.tile_pool(name="opool", bufs=4) as opool,
    ):
        sb_pools = dict(raw=rawpool, shuf=shufpool, vt=vtpool, p=ppool,
                        nf=nfpool, nb=nbpool, bc=bcpool, o=opool)
        def load_x_half(b, js, hh, eng=None, kog=KO):
            # kog < KO splits the transfer so compute can start on the
            # first ko chunks (used for the latency-critical halves)
            eng = eng or nc.sync
            x3 = x_t[b].rearrange("(ko p) t -> p ko t", p=128)
            hsl = slice(js * TSLAB + hh * 512, js * TSLAB + (hh + 1) * 512)
            x_h = xpool.tile([128, KO, 512], BF16, name="x_h")
            for kg in range(KO // kog):
                ks = slice(kg * kog, (kg + 1) * kog)
                eng.dma_start(x_h[:, ks, :], x3[:, ks, hsl])
            return x_h

        # Startup order matters: sync/scalar issue DMAs at ~1us each, so
        # the first-needed data leads on both queues (w chunk 0 of the
        # compute order + the two x halves of slab 0, split across
        # queues); consts and wp follow.
        w_sb = wqpool.tile([128, NCH, KO, 128], BF16)
        nc.scalar.dma_start(w_sb[:, CHUNK_ORDER[0], :, :],
                            w_qkv[:, CHUNK_ORDER[0], :, :])
        x00 = load_x_half(0, 0, 0, nc.sync, kog=KOG)
        x01 = load_x_half(0, 0, 1, nc.scalar, kog=KOG)
        # remaining W chunks ride the sync queue so the scalar queue is
        # free for the rope shuffle DMAs during qk compute
        for f in CHUNK_ORDER[1:]:
            nc.sync.dma_start(w_sb[:, f, :, :], w_qkv[:, f, :, :])
        ones_c_sb = consts.tile([128, 1], BF16)
        nc.sync.dma_start(ones_c_sb, ones_col)
        ones_r_sb = consts.tile([1, 128], BF16)
        nc.sync.dma_start(ones_r_sb, ones_row)
        tri_sb = consts.tile([128, 128], BF16)
        nc.sync.dma_start(tri_sb, tri)
        cos_sb = consts.tile([HD, T], BF16)
        nc.sync.dma_start(cos_sb, cos_t)
        sin_sb = consts.tile([HD, T], BF16)
        nc.sync.dma_start(sin_sb, sin_t)

        # Per-batch resident activations (reused across batches; the tile
        # framework serializes WAR hazards between batches automatically).
        qk_sb = qkres.tile([128, 8, T], BF16)  # chunks: q heads 0-3, k heads 4-7
        v_sb = vres.tile([128, T // 128, FV], BF16)  # natural [t, fv]
        y_sb = yres.tile([128, H_LOC, T], BF16)  # y^T per head

        # batch 0: slab-1 h0 prefetch (4th half waits on a slot)
        xq = [x00, x01, load_x_half(0, 1, 0)]

        for b in range(B_LOC):
            halves = {(0, 0): xq[0], (0, 1): xq[1], (1, 0): xq[2]}
            qkv_finish = _phase_qkv(nc, tc, b, halves, load_x_half, w_sb,
                                    cos_sb, sin_sb, qk_sb, v_sb, sb_pools)
            if b + 1 < B_LOC:
                # prefetch next batch's x during this batch's attention
                xq = [load_x_half(b + 1, 0, 0), load_x_half(b + 1, 0, 1),
                      load_x_half(b + 1, 1, 0)]
            _phase_attn(nc, tc, b, qk_sb, v_sb, y_sb, ones_c_sb, ones_r_sb,
                        tri_sb, sb_pools, qkv_finish)
            _phase_proj(nc, tc, b, wppool, w_proj, y_sb, out_t, sb_pools)


def _phase_qkv(nc, tc, b, halves, load_x_half, w_sb, cos_sb, sin_sb,
               qk_sb, v_sb, sb_pools):
    """qkv^T = W.T @ x^T in 128-feature chunks (v first, then q/k
    interleaved).  RoPE rotate-half via partition-strided SBUF->SBUF DMA:
    roped = raw*cos + shuf(raw)*sin_signed.  v chunks are evacuated as
    v^T and moved to natural [t, fv] layout with one wide DMA transpose
    per (chunk, slab).

    Emits all but the last two chunks, then returns a `finish` closure.
    The attention phase emits its first two score blocks before calling
    it, so their exps run on ACT while the PE chews the qkv tail."""
    import contextlib
    rawpool, shufpool, vtpool = (sb_pools["raw"], sb_pools["shuf"],
                                 sb_pools["vt"])
    es = contextlib.ExitStack()
    qkps = es.enter_context(tc.tile_pool(name="qkps", bufs=2, space="PSUM"))

    def emit_chunk(js, ci, f):
        tsl = slice(js * TSLAB, (js + 1) * TSLAB)
        h0 = halves[(js, 0)]
        h1 = halves[(js, 1)]
        ps = qkps.tile([128, TSLAB], F32, name="ps")
        # (1,1) arrives late: for the first chunks of slab 1 run the
        # halves unpaired so the PE chews h0 while h1 loads
        if js == 1 and ci < 2:
            for ko in range(KO):
                nc.tensor.matmul(ps[:, 0:512], w_sb[:, f, ko, :],
                                 h0[:, ko, :],
                                 start=(ko == 0), stop=(ko == KO - 1))
            for ko in range(KO):
                nc.tensor.matmul(ps[:, 512:], w_sb[:, f, ko, :],
                                 h1[:, ko, :],
                                 start=(ko == 0), stop=(ko == KO - 1))
        else:
            for ko in range(KO):
                nc.tensor.matmul(ps[:, 0:512], w_sb[:, f, ko, :],
                                 h0[:, ko, :],
                                 start=(ko == 0), stop=(ko == KO - 1))
                nc.tensor.matmul(ps[:, 512:], w_sb[:, f, ko, :],
                                 h1[:, ko, :],
                                 start=(ko == 0), stop=(ko == KO - 1))
        if f < 8:
            # q/k chunk: RoPE
            raw = rawpool.tile([128, TSLAB], BF16, name="raw")
            nc.scalar.copy(raw, ps)
            shuf = shufpool.tile([128, TSLAB], BF16, name="shuf")
            # rotate-half pair swap across adjacent partitions; issued
            # on the scalar queue right after the evac
            nc.scalar.dma_start(shuf[0:127:2, :], raw[1:128:2, :])
            nc.scalar.dma_start(shuf[1:128:2, :], raw[0:127:2, :])
            # t1 = raw*cos in place (Pool); t2 = shuf*sin_signed in
            # place (DVE); sum into the resident qk chunk
            nc.gpsimd.tensor_tensor(raw, raw, cos_sb[:, tsl], ALU.mult)
            nc.vector.tensor_tensor(shuf, shuf, sin_sb[:, tsl], ALU.mult)
            nc.vector.tensor_tensor(qk_sb[:, f, tsl], raw, shuf, ALU.add)
        else:
            # v chunk: evacuate v^T, wide-transpose into v_sb
            fc = f - 8
            vt = vtpool.tile([128, TSLAB], BF16, name="vt")
            nc.scalar.copy(vt, ps)
            nc.sync.dma_start_transpose(
                v_sb[:, js * (TSLAB // 128):(js + 1) * (TSLAB // 128),
                     fc * 128:(fc + 1) * 128],
                vt)

    for js in range(NSLAB):
        if js == 1 and (1, 1) not in halves:
            # slot for (1,1) frees once slab-0 compute is done
            halves[(1, 1)] = load_x_half(b, 1, 1, kog=KOG)
        for ci, f in enumerate(CHUNK_ORDER):
            if js == NSLAB - 1 and ci >= NCH - 2:
                continue  # deferred to finish()
            emit_chunk(js, ci, f)

    def finish():
        for ci in (NCH - 2, NCH - 1):
            emit_chunk(NSLAB - 1, ci, CHUNK_ORDER[ci])
        es.close()

    return finish


def _phase_attn(nc, tc, b, qk_sb, v_sb, y_sb, ones_c_sb, ones_r_sb, tri_sb,
                sb_pools, qkv_finish):
    """Causal attention per head, transposed orientation.
    scores^T [k, q] -> exp (single merged ACT instr) -> tri-mask (DVE) ->
    l (ones matmul), y^T = v_nat.T @ p^T; normalization via Pool
    partition broadcast."""
    ppool, nfpool, nbpool, bcpool = (sb_pools["p"], sb_pools["nf"],
                                     sb_pools["nb"], sb_pools["bc"])

    def jq_tail(ctx):
        """End-of-q-tile: finish evacuating y, 1/l per half, Pool
        partition broadcast, normalize."""
        h, q0, y_ps, l_ps = ctx
        nc.vector.tensor_copy(y_sb[:, h, q0 + 512:q0 + QT], y_ps[:, 512:])
        for hh in range(2):
            linv = nfpool.tile([1, 512], F32, name="linv")
            nc.vector.reciprocal_approx_fast(
                linv, l_ps[:, hh * 512:(hh + 1) * 512])
            linv_bf = nbpool.tile([1, 512], BF16, name="linv_bf")
            nc.vector.tensor_copy(linv_bf, linv)
            bc_sb = bcpool.tile([128, 512], BF16, name="bc_sb")
            nc.gpsimd.partition_broadcast(bc_sb, linv_bf)
            ysl = slice(q0 + hh * 512, q0 + (hh + 1) * 512)
            nc.vector.tensor_tensor(y_sb[:, h, ysl], y_sb[:, h, ysl],
                                    bc_sb, ALU.mult)

    import contextlib
    es = contextlib.ExitStack()
    sps = es.enter_context(tc.tile_pool(name="sps", bufs=2, space="PSUM", side="right"))

    def attn_block(blk):
        """Emit scores + exp + causal mask for one (h, jq, kb) block."""
        h, jq, kb = blk
        q0 = jq * QT
        qt = qk_sb[:, h, :]
        kt = qk_sb[:, 4 + h, :]
        s_diag = kb - NKBT * jq
        qoff = 128 * s_diag if s_diag > 0 else 0
        ksl = slice(kb * 128, (kb + 1) * 128)
        boff = max(0, qoff - 512)
        s_ps = sps.tile([128, QT], F32, name="s_ps")
        if qoff < 512:
            nc.tensor.matmul(
                s_ps[:, qoff:512], kt[:, ksl],
                qt[:, q0 + qoff:q0 + 512], start=True, stop=True)
        nc.tensor.matmul(
            s_ps[:, 512 + boff:], kt[:, ksl],
            qt[:, q0 + 512 + boff:q0 + QT], start=True, stop=True)
        p_sb = ppool.tile([128, QT], BF16, name="p_sb")
        nc.scalar.activation(p_sb[:, qoff:], s_ps[:, qoff:],
                             AF.Exp, scale=SCALE)
        if s_diag >= 0:
            # causal: zero p where q < k in the diagonal block
            nc.vector.tensor_tensor(
                p_sb[:, qoff:qoff + 128], p_sb[:, qoff:qoff + 128],
                tri_sb, ALU.mult)
        return (h, jq, kb, p_sb, qoff, boff)

    tiles = {}

    def drain(pb):
        """Emit l/pv for a previously emitted block; on half-A/q-tile
        completion, emit the deferred evacuation work."""
        h, jq, kb, p_sb, qoff, boff = pb
        q0 = jq * QT
        nkb = NKBT * (jq + 1)
        last_a = min(nkb - 1, NKBT * jq + 3)
        if kb == 0:
            tiles[(h, jq)] = (yps.tile([128, QT], F32, name="y_ps"),
                              lps.tile([1, QT], F32, name="l_ps"))
        y_ps, l_ps = tiles[(h, jq)]
        _emit_l_pv(nc, v_sb, ones_c_sb, h, l_ps, y_ps, last_a, nkb, p_sb,
                   kb, qoff, boff)
        if kb == last_a and last_a != nkb - 1:
            # half A complete: evacuate it early (frees the bank sooner)
            nc.vector.tensor_copy(y_sb[:, h, q0:q0 + 512], y_ps[:, 0:512])
        if kb == nkb - 1:
            jq_tail((h, q0, y_ps, l_ps))

    # flattened (h, jq, kb) stream with a 1-block skew that carries
    # across q-tile and head boundaries: scores(i+1) issue while exp(i)
    # runs on ACT, then l/pv(i) consume p(i).  The first two blocks are
    # emitted before the qkv tail so their exps hide under it.
    blocks = [(h, jq, kb)
              for h in range(H_LOC)
              for jq in range(NQT)
              for kb in range(NKBT * (jq + 1))]
    pending = [attn_block(blocks[0]), attn_block(blocks[1])]
    qkv_finish()
    yps = es.enter_context(tc.tile_pool(name="yps", bufs=1, space="PSUM", side="right"))
    lps = es.enter_context(tc.tile_pool(name="lps", bufs=1, space="PSUM", side="right"))
    drain(pending[0])
    prev = pending[1]
    for blk in blocks[2:]:
        cur = attn_block(blk)
        drain(prev)
        prev = cur
    drain(prev)
    es.close()


def _emit_l_pv(nc, v_sb, ones_c_sb, h, l_ps, y_ps, last_a, nkb, p_sb, kb,
               qoff, boff):
    """l += ones.T @ p ; y^T += v_nat.T @ p^T for one key block.
    Half A (q cols [0,512)) ends at last_a; half B at nkb-1."""
    hsl = slice(h * 128, (h + 1) * 128)
    if qoff < 512:
        nc.tensor.matmul(l_ps[:, qoff:512], ones_c_sb, p_sb[:, qoff:512],
                         start=(kb == 0), stop=(kb == last_a))
    nc.tensor.matmul(l_ps[:, 512 + boff:], ones_c_sb, p_sb[:, 512 + boff:],
                     start=(kb == 0), stop=(kb == nkb - 1))
    # one v lhsT load serves both halves
    if qoff < 512:
        nc.tensor.matmul(y_ps[:, qoff:512], v_sb[:, kb, hsl],
                         p_sb[:, qoff:512],
                         start=(kb == 0), stop=(kb == last_a))
    nc.tensor.matmul(y_ps[:, 512 + boff:], v_sb[:, kb, hsl],
                     p_sb[:, 512 + boff:],
                     start=(kb == 0), stop=(kb == nkb - 1))


def _phase_proj(nc, tc, b, wppool, w_proj, y_sb, out_t, sb_pools):
    """out^T[c, t] partial = Wp_loc.T @ y^T.  h-outer/jt-inner so one wp
    lhsT load serves 4 matmuls into 4 psum banks.  wp streamed per
    c-chunk (prefetched 3 deep) instead of held resident."""
    NJT = T // 512
    opool = sb_pools["o"]
    wp4 = w_proj.rearrange("(h p) c -> p h c", p=128)
    with tc.tile_pool(name="opsum", bufs=8, space="PSUM") as opsum:
        wq = []
        for co in range(3):
            wt = wppool.tile([128, H_LOC, 128], BF16, name="wp_sb")
            nc.sync.dma_start(wt, wp4[:, :, co * 128:(co + 1) * 128])
            wq.append(wt)
        for co in range(C // 128):
            if co + 3 < C // 128:
                wt = wppool.tile([128, H_LOC, 128], BF16, name="wp_sb")
                nc.sync.dma_start(
                    wt, wp4[:, :, (co + 3) * 128:(co + 4) * 128])
                wq.append(wt)
            wp_sb = wq[co]
            csl = slice(co * 128, (co + 1) * 128)
            o_ps = [opsum.tile([128, 512], F32, name="o_ps") for _ in range(NJT)]
            for h in range(H_LOC):
                for jt in range(NJT):
                    nc.tensor.matmul(
                        o_ps[jt], wp_sb[:, h, :],
                        y_sb[:, h, jt * 512:(jt + 1) * 512],
                        start=(h == 0), stop=(h == H_LOC - 1))
            for jt in range(NJT):
                o_sb = opool.tile([128, 512], BF16, name="o_sb")
                # alternate ACT/DVE for psum evacuation; the write rides
                # the matching queue to halve issue latency
                if jt % 2 == 0:
                    nc.scalar.copy(o_sb, o_ps[jt])
                    nc.scalar.dma_start(
                        out_t[b, csl, jt * 512:(jt + 1) * 512], o_sb)
                else:
                    nc.vector.tensor_copy(o_sb, o_ps[jt])
                    nc.sync.dma_start(
                        out_t[b, csl, jt * 512:(jt + 1) * 512], o_sb)


def _get_nc():
    if "nc" not in _CACHED:
        _CACHED["nc"] = build_nc()
    return _CACHED["nc"]


def kernel(x, sin, cos, W_qkv, W_proj):
    x = np.asarray(x, dtype=np.float32)
    sin = np.asarray(sin, dtype=np.float32)
    cos = np.asarray(cos, dtype=np.float32)
    W_qkv = np.asarray(W_qkv, dtype=np.float32)
    W_proj = np.asarray(W_proj, dtype=np.float32)

    # rotate-half is a pure pair swap on chip; the sign lives in sin:
    # roped[2i] = raw[2i]cos - raw[2i+1]sin ; roped[2i+1] = raw[2i+1]cos
    # + raw[2i]sin  =>  sin row 2i negated.
    sin_tn = np.ascontiguousarray(sin[0, 0].T).copy()  # [HD, T]
    sin_tn[0::2, :] *= -1.0
    sin_t = sin_tn.astype(NPBF)
    cos_t = np.ascontiguousarray(cos[0, 0].T).astype(NPBF)
    ones_col = np.ones((128, 1), NPBF)
    ones_row = np.ones((1, 128), NPBF)
    tri = np.triu(np.ones((128, 128), np.float32)).astype(NPBF)

    in_maps = []
    for g in range(BGROUPS):
        x_tg = np.ascontiguousarray(
            x[g * B_LOC:(g + 1) * B_LOC].transpose(0, 2, 1)
        ).astype(NPBF)  # [B_LOC, C, T]
        for s in range(HSHARDS):
            qcols = W_qkv[:, s * FQK:(s + 1) * FQK]
            kcols = W_qkv[:, C + s * FQK:C + (s + 1) * FQK]
            vcols = W_qkv[:, 2 * C + s * FV:2 * C + (s + 1) * FV]
            w_flat = np.concatenate([qcols, kcols, vcols], axis=1)
            # [C, 1536] -> [p, chunk, ko, f] with C = ko*128 + p
            w_qkv_loc = np.ascontiguousarray(
                w_flat.reshape(KO, 128, NCH, 128).transpose(1, 2, 0, 3)
            ).astype(NPBF)
            w_proj_loc = np.ascontiguousarray(
                W_proj[s * FV:(s + 1) * FV, :]).astype(NPBF)
            in_maps.append(
                {
                    "x_t": x_tg,
                    "w_qkv": w_qkv_loc,
                    "w_proj": w_proj_loc,
                    "sin_t": sin_t,
                    "cos_t": cos_t,
                    "ones_col": ones_col,
                    "ones_row": ones_row,
                    "tri": tri,
                }
            )

    trace = bool(int(os.environ.get("KERNEL_TRACE", "0")))
    if trace:
        _install_ntff_hook()
    nc = _get_nc()
    res = run_bass_kernel_spmd(
        nc, in_maps, core_ids=list(range(NCORES)), trace=trace
    )
    _CACHED["last_result"] = res

    out = np.zeros((B, T, C), dtype=np.float32)
    for g in range(BGROUPS):
        acc = np.zeros((B_LOC, C, T), dtype=np.float32)
        for s in range(HSHARDS):
            acc += res.results[g * HSHARDS + s]["out_t"].astype(np.float32)
        out[g * B_LOC:(g + 1) * B_LOC] = acc.transpose(0, 2, 1)
    return out


# revision 31
# speedup vs baseline: 1.0263x; 1.0263x over previous
"""Trainium2 Bass kernel for nn_MHA_43095701848407.

MHA forward: qkv = x @ W_qkv, RoPE on q/k, causal softmax attention,
y @ W_proj.  B=4, T=2048, C=2048, 16 heads, head_dim=128.

Sharding (8 cores): tensor-parallel over heads (4 shards x 4 heads) x
data-parallel over batch (2 groups x 2 batches).  core = group*4 + shard.

Design notes (measured on hw): PE wall per matmul is ~N/2.4GHz + 3ns
with bf16 operands (weight loads fully hidden by the background weight
buffer; fp32r pays ~70ns extra per load), so the kernel minimizes
streamed PE columns and instruction count, and keeps every other engine
off the PE critical path:
- all matmul operands bf16 (abundant tolerance: gate 2e-2, measured 5e-3)
- q^T/k^T/v SBUF-resident per batch; per-batch pipeline qkv->attn->proj
  with x/W prefetch on both DMA-issue queues (sync + scalar)
- W_qkv host-relaid to [p, chunk, ko, f] for 4KB-contiguous DMA runs
- RoPE rotate-half as a partition-strided SBUF->SBUF DMA pair (off PE);
  the sign folded into sin on the host
- v computed transposed in the W-stream, moved to natural layout by one
  wide DMA-transpose per (chunk, slab); v chunks run before q/k so the
  transposes drain during qk compute
- attention flattened over (head, q-tile, key-block) with a 1-block
  skew carried across boundaries; single merged exp per key block from
  a 2-bank PSUM tile; causal mask via triangular-mask multiply on DVE
- softmax 1/l broadcast via gpsimd partition_broadcast (no PE/PSUM)
- proj h-outer/jt-inner for lhsT reuse, wp streamed per c-chunk,
  out writes split across both DMA queues
Host sums the 4 head-shard bf16 partials per batch in f32.

Self-contained: shapes/sharding hardcoded; inputs full-size numpy arrays.
"""

import math
import os
import sys
import types

import ml_dtypes
import numpy as np

import concourse.bass as bass
import concourse.mybir as mybir
import concourse.tile as tile
from concourse import bacc
from concourse.bass_utils import run_bass_kernel_spmd

F32 = mybir.dt.float32
BF16 = mybir.dt.bfloat16
AF = mybir.ActivationFunctionType
ALU = mybir.AluOpType
NPBF = ml_dtypes.bfloat16

# Problem shape (hardcoded per contract)
B, T, C = 4, 2048, 2048
H, HD = 16, 128
NCORES = 8
BGROUPS, HSHARDS = 2, 4  # batch groups x head shards
B_LOC = B // BGROUPS  # 2 batches per core
H_LOC = H // HSHARDS  # 4 heads per core
FQK = H_LOC * HD  # 512 features for q (and for k)
FV = H_LOC * HD  # 512 for v
NCH = 12  # qkv feature chunks of 128 (4 q + 4 k + 4 v)
# v chunks (8..11) first so their DMA transposes drain during qk compute;
# qk interleaved q0,k0,q1,k1,... so early heads are ready first.
CHUNK_ORDER = [8, 9, 10, 11, 0, 4, 1, 5, 2, 6, 3, 7]
KO = C // 128  # 16 contraction chunks
KOG = 4  # x DMA granularity: 4 ko chunks per transfer
TSLAB = 1024
NSLAB = T // TSLAB  # 2 t-slabs per batch
QT = 1024  # attention q tile
NQT = T // QT  # 2 q tiles
NKBT = QT // 128  # 8 key blocks per q tile width
SCALE = 1.0 / math.sqrt(HD)

_CACHED = {}


def _install_ntff_hook():
    """Register the axon NTFF profile hook (container's antenv lacks it)."""
    if "antenv.axon_hooks" in sys.modules:
        return
    try:
        mod = types.ModuleType("antenv.axon_hooks")
        holder = [None]
        mod.set_axon_ntff_profile_hook = lambda h: holder.__setitem__(0, h)
        mod.get_axon_ntff_profile_hook = lambda: holder[0]
        sys.modules["antenv.axon_hooks"] = mod
        import antenv

        antenv.axon_hooks = mod
        if "/root/.axon_site" not in sys.path:
            sys.path.insert(0, "/root/.axon_site")
        from trn_agent_boot.trn_boot import _ntff_profile_via_ctypes

        mod.set_axon_ntff_profile_hook(
            _ntff_profile_via_ctypes("/opt/axon/libaxon_pjrt.so")
        )
    except Exception:
        sys.modules.pop("antenv.axon_hooks", None)


def build_nc():
    nc = bacc.Bacc("TRN2", target_bir_lowering=False, debug=False)

    x_t = nc.dram_tensor("x_t", [B_LOC, C, T], BF16, kind="ExternalInput").ap()
    w_qkv = nc.dram_tensor("w_qkv", [128, NCH, KO, 128], BF16,
                           kind="ExternalInput").ap()
    w_proj = nc.dram_tensor("w_proj", [FV, C], BF16, kind="ExternalInput").ap()
    cos_t = nc.dram_tensor("cos_t", [HD, T], BF16, kind="ExternalInput").ap()
    sin_t = nc.dram_tensor("sin_t", [HD, T], BF16, kind="ExternalInput").ap()
    ones_col = nc.dram_tensor("ones_col", [128, 1], BF16, kind="ExternalInput").ap()
    ones_row = nc.dram_tensor("ones_row", [1, 128], BF16, kind="ExternalInput").ap()
    tri = nc.dram_tensor("tri", [128, 128], BF16, kind="ExternalInput").ap()
    out_t = nc.dram_tensor("out_t", [B_LOC, C, T], BF16, kind="ExternalOutput").ap()

    with tile.TileContext(nc) as tc:
        with nc.allow_low_precision(reason="bf16 matmuls by design; tol 2e-2"):
            _emit(nc, tc, x_t, w_qkv, w_proj, cos_t, sin_t, ones_col,
                  ones_row, tri, out_t)
    nc.compile()
    return nc


def _emit(nc, tc, x_t, w_qkv, w_proj, cos_t, sin_t, ones_col, ones_row,
          tri, out_t):
    with (
        tc.tile_pool(name="consts", bufs=1) as consts,
        tc.tile_pool(name="wq", bufs=1) as wqpool,
        tc.tile_pool(name="wp", bufs=4) as wppool,
        tc.tile_pool(name="qkres", bufs=1) as qkres,
        tc.tile_pool(name="vres", bufs=1) as vres,
        tc.tile_pool(name="yres", bufs=1) as yres,
        tc.tile_pool(name="xpool", bufs=3) as xpool,
        tc.tile_pool(name="rawpool", bufs=2) as rawpool,
        tc.tile_pool(name="shufpool", bufs=2) as shufpool,
        tc.tile_pool(name="vtpool", bufs=2) as vtpool,
        tc.tile_pool(name="ppool", bufs=4) as ppool,
        tc.tile_pool(name="nfpool", bufs=1) as nfpool,
        tc.tile_pool(name="nbpool", bufs=1) as nbpool,
        tc.tile_pool(name="bcpool", bufs=2) as bcpool,
        tc.tile_pool(name="opool", bufs=4) as opool,
    ):
        sb_pools = dict(raw=rawpool, shuf=shufpool, vt=vtpool, p=ppool,
                        nf=nfpool, nb=nbpool, bc=bcpool, o=opool)
        def load_x_half(b, js, hh, eng=None, kog=KO):
            # kog < KO splits the transfer so compute can start on the
            # first ko chunks (used for the latency-critical halves)
            eng = eng or nc.sync
            x3 = x_t[b].rearrange("(ko p) t -> p ko t", p=128)
            hsl = slice(js * TSLAB + hh * 512, js * TSLAB + (hh + 1) * 512)
            x_h = xpool.tile([128, KO, 512], BF16, name="x_h")
            for kg in range(KO // kog):
                ks = slice(kg * kog, (kg + 1) * kog)
                eng.dma_start(x_h[:, ks, :], x3[:, ks, hsl])
            return x_h

        # Startup order matters: sync/scalar issue DMAs at ~1us each, so
        # the first-needed data leads on both queues (w chunk 0 of the
        # compute order + the two x halves of slab 0, split across
        # queues); consts and wp follow.
        w_sb = wqpool.tile([128, NCH, KO, 128], BF16)
        nc.scalar.dma_start(w_sb[:, CHUNK_ORDER[0], :, :],
                            w_qkv[:, CHUNK_ORDER[0], :, :])
        x00 = load_x_half(0, 0, 0, nc.sync, kog=KOG)
        x01 = load_x_half(0, 0, 1, nc.scalar, kog=KOG)
        # remaining W chunks ride the sync queue so the scalar queue is
        # free for the rope shuffle DMAs during qk compute
        for f in CHUNK_ORDER[1:]:
            nc.sync.dma_start(w_sb[:, f, :, :], w_qkv[:, f, :, :])
        ones_c_sb = consts.tile([128, 1], BF16)
        nc.sync.dma_start(ones_c_sb, ones_col)
        ones_r_sb = consts.tile([1, 128], BF16)
        nc.sync.dma_start(ones_r_sb, ones_row)
        tri_sb = consts.tile([128, 128], BF16)
        nc.sync.dma_start(tri_sb, tri)
        cos_sb = consts.tile([HD, T], BF16)
        nc.sync.dma_start(cos_sb, cos_t)
        sin_sb = consts.tile([HD, T], BF16)
        nc.sync.dma_start(sin_sb, sin_t)

        # Per-batch resident activations (reused across batches; the tile
        # framework serializes WAR hazards between batches automatically).
        qk_sb = qkres.tile([128, 8, T], BF16)  # chunks: q heads 0-3, k heads 4-7
        v_sb = vres.tile([128, T // 128, FV], BF16)  # natural [t, fv]
        y_sb = yres.tile([128, H_LOC, T], BF16)  # y^T per head

        # batch 0: slab-1 h0 prefetch (4th half waits on a slot)
        xq = [x00, x01, load_x_half(0, 1, 0)]

        for b in range(B_LOC):
            halves = {(0, 0): xq[0], (0, 1): xq[1], (1, 0): xq[2]}
            _phase_qkv(nc, tc, b, halves, load_x_half, w_sb, cos_sb, sin_sb,
                       qk_sb, v_sb, sb_pools)
            if b + 1 < B_LOC:
                # prefetch next batch's x during this batch's attention
                xq = [load_x_half(b + 1, 0, 0), load_x_half(b + 1, 0, 1),
                      load_x_half(b + 1, 1, 0)]
            _phase_attn(nc, tc, b, qk_sb, v_sb, y_sb, ones_c_sb, ones_r_sb,
                        tri_sb, sb_pools)
            _phase_proj(nc, tc, b, wppool, w_proj, y_sb, out_t, sb_pools)


def _phase_qkv(nc, tc, b, halves, load_x_half, w_sb, cos_sb, sin_sb,
               qk_sb, v_sb, sb_pools):
    """qkv^T = W.T @ x^T in 128-feature chunks (v first, then q/k
    interleaved).  RoPE rotate-half via partition-strided SBUF->SBUF DMA:
    roped = raw*cos + shuf(raw)*sin_signed.  v chunks are evacuated as
    v^T and moved to natural [t, fv] layout with one wide DMA transpose
    per (chunk, slab)."""
    rawpool, shufpool, vtpool = (sb_pools["raw"], sb_pools["shuf"],
                                 sb_pools["vt"])
    with tc.tile_pool(name="qkps", bufs=3, space="PSUM") as qkps:
        for js in range(NSLAB):
            tsl = slice(js * TSLAB, (js + 1) * TSLAB)
            if js == 1 and (1, 1) not in halves:
                # slot for (1,1) frees once slab-0 compute is done
                halves[(1, 1)] = load_x_half(b, 1, 1, kog=KOG)
            h0 = halves[(js, 0)]
            h1 = halves[(js, 1)]
            for ci, f in enumerate(CHUNK_ORDER):
                ps = qkps.tile([128, TSLAB], F32, name="ps")
                # (1,1) arrives late: for the first chunk of slab 1 run
                # the halves unpaired so the PE chews h0 while h1 loads
                if js == 1 and ci < 2:
                    for ko in range(KO):
                        nc.tensor.matmul(ps[:, 0:512], w_sb[:, f, ko, :],
                                         h0[:, ko, :],
                                         start=(ko == 0), stop=(ko == KO - 1))
                    for ko in range(KO):
                        nc.tensor.matmul(ps[:, 512:], w_sb[:, f, ko, :],
                                         h1[:, ko, :],
                                         start=(ko == 0), stop=(ko == KO - 1))
                else:
                    for ko in range(KO):
                        nc.tensor.matmul(ps[:, 0:512], w_sb[:, f, ko, :],
                                         h0[:, ko, :],
                                         start=(ko == 0), stop=(ko == KO - 1))
                        nc.tensor.matmul(ps[:, 512:], w_sb[:, f, ko, :],
                                         h1[:, ko, :],
                                         start=(ko == 0), stop=(ko == KO - 1))
                if f < 8:
                    # q/k chunk: RoPE
                    raw = rawpool.tile([128, TSLAB], BF16, name="raw")
                    nc.scalar.copy(raw, ps)
                    shuf = shufpool.tile([128, TSLAB], BF16, name="shuf")
                    # rotate-half pair swap across adjacent partitions;
                    # issued on the scalar queue right after the evac
                    nc.scalar.dma_start(shuf[0:127:2, :], raw[1:128:2, :])
                    nc.scalar.dma_start(shuf[1:128:2, :], raw[0:127:2, :])
                    # t1 = raw*cos in place (Pool); t2 = shuf*sin_signed
                    # in place (DVE); sum into the resident qk chunk
                    nc.gpsimd.tensor_tensor(raw, raw, cos_sb[:, tsl], ALU.mult)
                    nc.vector.tensor_tensor(shuf, shuf, sin_sb[:, tsl],
                                            ALU.mult)
                    nc.vector.tensor_tensor(qk_sb[:, f, tsl], raw, shuf,
                                            ALU.add)
                else:
                    # v chunk: evacuate v^T, wide-transpose into v_sb
                    fc = f - 8
                    vt = vtpool.tile([128, TSLAB], BF16, name="vt")
                    nc.scalar.copy(vt, ps)
                    nc.sync.dma_start_transpose(
                        v_sb[:, js * (TSLAB // 128):(js + 1) * (TSLAB // 128),
                             fc * 128:(fc + 1) * 128],
                        vt)


def _phase_attn(nc, tc, b, qk_sb, v_sb, y_sb, ones_c_sb, ones_r_sb, tri_sb,
                sb_pools):
    """Causal attention per head, transposed orientation.
    scores^T [k, q] -> exp (single merged ACT instr) -> tri-mask (DVE) ->
    l (ones matmul), y^T = v_nat.T @ p^T; normalization via Pool
    partition broadcast."""
    ppool, nfpool, nbpool, bcpool = (sb_pools["p"], sb_pools["nf"],
                                     sb_pools["nb"], sb_pools["bc"])

    def jq_tail(ctx):
        """End-of-q-tile: finish evacuating y, 1/l per half, Pool
        partition broadcast, normalize."""
        h, q0, y_ps, l_ps = ctx
        nc.vector.tensor_copy(y_sb[:, h, q0 + 512:q0 + QT], y_ps[:, 512:])
        for hh in range(2):
            linv = nfpool.tile([1, 512], F32, name="linv")
            nc.vector.reciprocal_approx_fast(
                linv, l_ps[:, hh * 512:(hh + 1) * 512])
            linv_bf = nbpool.tile([1, 512], BF16, name="linv_bf")
            nc.vector.tensor_copy(linv_bf, linv)
            bc_sb = bcpool.tile([128, 512], BF16, name="bc_sb")
            nc.gpsimd.partition_broadcast(bc_sb, linv_bf)
            ysl = slice(q0 + hh * 512, q0 + (hh + 1) * 512)
            nc.vector.tensor_tensor(y_sb[:, h, ysl], y_sb[:, h, ysl],
                                    bc_sb, ALU.mult)

    with (
        tc.tile_pool(name="sps", bufs=2, space="PSUM") as sps,
        tc.tile_pool(name="yps", bufs=1, space="PSUM") as yps,
        tc.tile_pool(name="lps", bufs=1, space="PSUM") as lps,
    ):
        # flattened (h, jq, kb) stream with a 1-block skew that carries
        # across q-tile and head boundaries: scores(i+1) issues while
        # exp(i) runs on ACT, then l/pv(i) consume p(i)
        blocks = [(h, jq, kb)
                  for h in range(H_LOC)
                  for jq in range(NQT)
                  for kb in range(NKBT * (jq + 1))]
        prev = None   # (l_pv args..., ctx, is_last)
        tiles = {}
        for (h, jq, kb) in blocks:
            q0 = jq * QT
            nkb = NKBT * (jq + 1)
            last_a = min(nkb - 1, NKBT * jq + 3)
            if kb == 0:
                tiles[(h, jq)] = (yps.tile([128, QT], F32, name="y_ps"),
                                  lps.tile([1, QT], F32, name="l_ps"))
            y_ps, l_ps = tiles[(h, jq)]
            qt = qk_sb[:, h, :]
            kt = qk_sb[:, 4 + h, :]
            s_diag = kb - NKBT * jq
            qoff = 128 * s_diag if s_diag > 0 else 0
            ksl = slice(kb * 128, (kb + 1) * 128)
            boff = max(0, qoff - 512)
            s_ps = sps.tile([128, QT], F32, name="s_ps")
            if qoff < 512:
                nc.tensor.matmul(
                    s_ps[:, qoff:512], kt[:, ksl],
                    qt[:, q0 + qoff:q0 + 512], start=True, stop=True)
            nc.tensor.matmul(
                s_ps[:, 512 + boff:], kt[:, ksl],
                qt[:, q0 + 512 + boff:q0 + QT], start=True, stop=True)
            p_sb = ppool.tile([128, QT], BF16, name="p_sb")
            nc.scalar.activation(p_sb[:, qoff:], s_ps[:, qoff:],
                                 AF.Exp, scale=SCALE)
            if s_diag >= 0:
                # causal: zero p where q < k in the diagonal block
                nc.vector.tensor_tensor(
                    p_sb[:, qoff:qoff + 128], p_sb[:, qoff:qoff + 128],
                    tri_sb, ALU.mult)
            if prev is not None:
                _drain(nc, v_sb, ones_c_sb, y_sb, jq_tail, prev)
            prev = (h, l_ps, y_ps, last_a, nkb, p_sb, kb, qoff, boff,
                    (h, q0, y_ps, l_ps))
        _drain(nc, v_sb, ones_c_sb, y_sb, jq_tail, prev)


def _drain(nc, v_sb, ones_c_sb, y_sb, jq_tail, prev):
    """Emit l/pv for the previous block; on half-A/q-tile completion,
    emit the deferred evacuation work."""
    (h, l_ps, y_ps, last_a, nkb, p_sb, kb, qoff, boff, ctx) = prev
    _emit_l_pv(nc, v_sb, ones_c_sb, h, l_ps, y_ps, last_a, nkb, p_sb, kb,
               qoff, boff)
    if kb == last_a and last_a != nkb - 1:
        # half A complete: evacuate it early (frees the bank sooner)
        q0 = ctx[1]
        nc.vector.tensor_copy(y_sb[:, h, q0:q0 + 512], y_ps[:, 0:512])
    if kb == nkb - 1:
        jq_tail(ctx)


def _emit_l_pv(nc, v_sb, ones_c_sb, h, l_ps, y_ps, last_a, nkb, p_sb, kb,
               qoff, boff):
    """l += ones.T @ p ; y^T += v_nat.T @ p^T for one key block.
    Half A (q cols [0,512)) ends at last_a; half B at nkb-1."""
    hsl = slice(h * 128, (h + 1) * 128)
    if qoff < 512:
        nc.tensor.matmul(l_ps[:, qoff:512], ones_c_sb, p_sb[:, qoff:512],
                         start=(kb == 0), stop=(kb == last_a))
    nc.tensor.matmul(l_ps[:, 512 + boff:], ones_c_sb, p_sb[:, 512 + boff:],
                     start=(kb == 0), stop=(kb == nkb - 1))
    # one v lhsT load serves both halves
    if qoff < 512:
        nc.tensor.matmul(y_ps[:, qoff:512], v_sb[:, kb, hsl],
                         p_sb[:, qoff:512],
                         start=(kb == 0), stop=(kb == last_a))
    nc.tensor.matmul(y_ps[:, 512 + boff:], v_sb[:, kb, hsl],
                     p_sb[:, 512 + boff:],
                     start=(kb == 0), stop=(kb == nkb - 1))


def _phase_proj(nc, tc, b, wppool, w_proj, y_sb, out_t, sb_pools):
    """out^T[c, t] partial = Wp_loc.T @ y^T.  h-outer/jt-inner so one wp
    lhsT load serves 4 matmuls into 4 psum banks.  wp streamed per
    c-chunk (prefetched 3 deep) instead of held resident."""
    NJT = T // 512
    opool = sb_pools["o"]
    wp4 = w_proj.rearrange("(h p) c -> p h c", p=128)
    with tc.tile_pool(name="opsum", bufs=8, space="PSUM") as opsum:
        wq = []
        for co in range(3):
            wt = wppool.tile([128, H_LOC, 128], BF16, name="wp_sb")
            nc.sync.dma_start(wt, wp4[:, :, co * 128:(co + 1) * 128])
            wq.append(wt)
        for co in range(C // 128):
            if co + 3 < C // 128:
                wt = wppool.tile([128, H_LOC, 128], BF16, name="wp_sb")
                nc.sync.dma_start(
                    wt, wp4[:, :, (co + 3) * 128:(co + 4) * 128])
                wq.append(wt)
            wp_sb = wq[co]
            csl = slice(co * 128, (co + 1) * 128)
            o_ps = [opsum.tile([128, 512], F32, name="o_ps") for _ in range(NJT)]
            for h in range(H_LOC):
                for jt in range(NJT):
                    nc.tensor.matmul(
                        o_ps[jt], wp_sb[:, h, :],
                        y_sb[:, h, jt * 512:(jt + 1) * 512],
                        start=(h == 0), stop=(h == H_LOC - 1))
            for jt in range(NJT):
                o_sb = opool.tile([128, 512], BF16, name="o_sb")
                # alternate ACT/DVE for psum evacuation; the write rides
                # the matching queue to halve issue latency
                if jt % 2 == 0:
                    nc.scalar.copy(o_sb, o_ps[jt])
                    nc.scalar.dma_start(
                        out_t[b, csl, jt * 512:(jt + 1) * 512], o_sb)
                else:
                    nc.vector.tensor_copy(o_sb, o_ps[jt])
                    nc.sync.dma_start(
                        out_t[b, csl, jt * 512:(jt + 1) * 512], o_sb)


def _get_nc():
    if "nc" not in _CACHED:
        _CACHED["nc"] = build_nc()
    return _CACHED["nc"]


def kernel(x, sin, cos, W_qkv, W_proj):
    x = np.asarray(x, dtype=np.float32)
    sin = np.asarray(sin, dtype=np.float32)
    cos = np.asarray(cos, dtype=np.float32)
    W_qkv = np.asarray(W_qkv, dtype=np.float32)
    W_proj = np.asarray(W_proj, dtype=np.float32)

    # rotate-half is a pure pair swap on chip; the sign lives in sin:
    # roped[2i] = raw[2i]cos - raw[2i+1]sin ; roped[2i+1] = raw[2i+1]cos
    # + raw[2i]sin  =>  sin row 2i negated.
    sin_tn = np.ascontiguousarray(sin[0, 0].T).copy()  # [HD, T]
    sin_tn[0::2, :] *= -1.0
    sin_t = sin_tn.astype(NPBF)
    cos_t = np.ascontiguousarray(cos[0, 0].T).astype(NPBF)
    ones_col = np.ones((128, 1), NPBF)
    ones_row = np.ones((1, 128), NPBF)
    tri = np.triu(np.ones((128, 128), np.float32)).astype(NPBF)

    in_maps = []
    for g in range(BGROUPS):
        x_tg = np.ascontiguousarray(
            x[g * B_LOC:(g + 1) * B_LOC].transpose(0, 2, 1)
        ).astype(NPBF)  # [B_LOC, C, T]
        for s in range(HSHARDS):
            qcols = W_qkv[:, s * FQK:(s + 1) * FQK]
            kcols = W_qkv[:, C + s * FQK:C + (s + 1) * FQK]
            vcols = W_qkv[:, 2 * C + s * FV:2 * C + (s + 1) * FV]
            w_flat = np.concatenate([qcols, kcols, vcols], axis=1)
            # [C, 1536] -> [p, chunk, ko, f] with C = ko*128 + p
            w_qkv_loc = np.ascontiguousarray(
                w_flat.reshape(KO, 128, NCH, 128).transpose(1, 2, 0, 3)
            ).astype(NPBF)
            w_proj_loc = np.ascontiguousarray(
                W_proj[s * FV:(s + 1) * FV, :]).astype(NPBF)
            in_maps.append(
                {
                    "x_t": x_tg,
                    "w_qkv": w_qkv_loc,
                    "w_proj": w_proj_loc,
                    "sin_t": sin_t,
                    "cos_t": cos_t,
                    "ones_col": ones_col,
                    "ones_row": ones_row,
                    "tri": tri,
                }
            )

    trace = bool(int(os.environ.get("KERNEL_TRACE", "0")))
    if trace:
        _install_ntff_hook()
    nc = _get_nc()
    res = run_bass_kernel_spmd(
        nc, in_maps, core_ids=list(range(NCORES)), trace=trace
    )
    _CACHED["last_result"] = res

    out = np.zeros((B, T, C), dtype=np.float32)
    for g in range(BGROUPS):
        acc = np.zeros((B_LOC, C, T), dtype=np.float32)
        for s in range(HSHARDS):
            acc += res.results[g * HSHARDS + s]["out_t"].astype(np.float32)
        out[g * B_LOC:(g + 1) * B_LOC] = acc.transpose(0, 2, 1)
    return out
